# revision 11
# baseline (speedup 1.0000x reference)
"""
nn_BiReBlock kernel for 8x Trainium2 NeuronCores.

Mathematical reduction (same as the verified baseline)
------------------------------------------------------
reference(X, W) with W having orthonormal rows reduces to
    out = Wm @ X @ Wm^T + eps * diag(1_N)
where Wm = W with QR-sign-negative rows zeroed (for the actual seed-0 W,
QR reproduces W exactly so Wm = W, N = {}).

Device computation (v6, "packed half-staircase")
------------------------------------------------
The kernel is HBM-DMA-bound, so we ship as few bytes as possible:

* fp16 is plenty (2e-2 budget vs ~4e-4 measured end-to-end), no residual.
* X is symmetric, so only the lower 2x2 block staircase of it is shipped:
  L' = [[X11/2, 0], [X21, X22/2]], which satisfies L' + L'^T = X.
  Since S = Wm X Wm^T is symmetric the device computes
      Z_b = Wm L'_b^T Wm^T
  and the host reconstructs S = Z + Z^T for free.  75% of X shipped.
* The left block-column [128, 64] is one dense SBUF tile per item.
  The right-bottom block [64, 64] of consecutive item pairs is packed
  onto all 128 SBUF partitions (even item on 0:64, odd on 64:128), so
  every DMA runs with full 128-partition parallelism and no SBUF zeros
  are ever needed.  Stage 1 = 2 column-tiled matmuls per item:
    V[0:64]  = (left col)^T W^T        K=128, tile (0,0)
    V[64:128]= (X22/2)^T  W^T[64:128]  K=64,  tile (0|64, 64)
  where the odd-item MM contracts on partitions 64:128 against the
  resident W^T rows, and the even-item MM contracts on partitions 0:64
  against a shifted copy of W^T rows 64:128 (third section of WH3).
* stage 2 packs two 8-item groups into PSUM partitions 0:64 / 64:128
  via column tiling; it is emitted one pair late (software pipelining)
  so its wait for the PSUM->SBUF copy never stalls the in-order PE.
* PSUM->SBUF copies convert to fp16 and are split/alternated across
  the Vector and Scalar engines; output is fp16.

HBM traffic/core: 12 MB X + 4 MB out (vs 32 MB baseline).
"""

import numpy as np

B_TOTAL = 4096
N_CORES = 8
B_LOCAL = B_TOTAL // N_CORES
D_IN = 128
D_OUT = 64
EPS = 1e-4

_CACHE = {}

CHUNKS = [32, 32] + [64] * 7
assert sum(CHUNKS) == B_LOCAL
XCH_MAX = max(CHUNKS)
NXBUF = 6
GROUP = 8
PAIR = 2 * GROUP          # items per packed stage-2 PSUM bank
OCH = 64                  # items per output flush
H = D_IN // 2             # 64


def _build_nc(b_local):
    import concourse.tile as tile
    from concourse import bacc, mybir

    f32 = mybir.dt.float32
    f16 = mybir.dt.float16
    nc = bacc.Bacc(None, target_bir_lowering=False)

    # left block-column of L', i-major: XL[i, b*64 + j] = L'_b[i, j]
    xld = nc.dram_tensor("XL", [D_IN, b_local * H], f16, kind="ExternalInput")
    # right-bottom blocks, item-pair packed:
    # XR[s*64 + p, q*64 + j] = (X22/2)_{2q+s}[p, j]
    xrd = nc.dram_tensor("XR", [D_IN, (b_local // 2) * H], f16,
                         kind="ExternalInput")
    # [W^T | W^T | W^T[64:128] duplicated on both partition halves]
    wd = nc.dram_tensor("WH3", [D_IN, 3 * D_OUT], f16, kind="ExternalInput")
    n_pair = b_local // PAIR
    outd = nc.dram_tensor("OUT", [D_IN, n_pair * GROUP * D_OUT], f16,
                          kind="ExternalOutput")

    gfree = GROUP * D_OUT   # 512
    pfree = PAIR * D_OUT    # 1024

    with tile.TileContext(nc) as tc:
        with (
            tc.tile_pool(name="const", bufs=1) as cpool,
            tc.tile_pool(name="tsb", bufs=3) as tpool,
            tc.tile_pool(name="obuf", bufs=2) as opool,
            tc.tile_pool(name="psum_t", bufs=3, space="PSUM") as pt,
            tc.tile_pool(name="psum_s", bufs=2, space="PSUM") as ps,
        ):
            wh3 = cpool.tile([D_IN, 3 * D_OUT], f16)
            nc.sync.dma_start(wh3[:], wd[:])

            # fixed X tiles (NXBUF buffers x {left, right}), rotated manually
            xbufs = [
                (cpool.tile([D_IN, XCH_MAX, H], f16, name=f"xl{i}"),
                 cpool.tile([D_IN, XCH_MAX // 2, H], f16, name=f"xr{i}"))
                for i in range(NXBUF)
            ]

            chunk_base = np.cumsum([0] + CHUNKS).tolist()

            def issue_chunk(k):
                b0, n = chunk_base[k], CHUNKS[k]
                xl, xr = xbufs[k % NXBUF]
                nc.sync.dma_start(
                    xl[:, 0:n, :], xld[:, b0 * H : (b0 + n) * H]
                )
                nc.sync.dma_start(
                    xr[:, 0 : n // 2, :],
                    xrd[:, (b0 // 2) * H : ((b0 + n) // 2) * H],
                )

            # chunk k+NXBUF-1 is issued only once chunk k's compute emission
            # begins, so WAR edges against the buffer's previous user are
            # ordered correctly.
            for k in range(min(NXBUF - 1, len(CHUNKS))):
                issue_chunk(k)

            obufs = {}

            def emit_stage2(pg, ts):
                sp = ps.tile([D_IN, gfree], f32, tag="sp", name="sp")
                for h in range(2):
                    nc.tensor.matmul(
                        sp[h * D_OUT : (h + 1) * D_OUT, :],
                        wh3[:, h * D_OUT : (h + 1) * D_OUT],
                        ts[:, h * gfree : (h + 1) * gfree],
                        start=True,
                        stop=True,
                        tile_position=(0, h * D_OUT),
                    )
                obuf = obufs[pg // (OCH // PAIR)]
                off = (pg % (OCH // PAIR)) * gfree
                scpy = nc.scalar.copy if pg % 2 == 0 else nc.vector.tensor_copy
                scpy(obuf[:, off : off + gfree], sp[:])
                c0 = pg * PAIR
                if (c0 + PAIR) % OCH == 0:
                    o0 = (pg // (OCH // PAIR)) * (OCH // PAIR) * gfree
                    olen = (OCH // PAIR) * gfree
                    if c0 + PAIR == b_local:
                        nc.gpsimd.dma_start(
                            outd[:, o0 : o0 + olen // 2], obuf[:, : olen // 2]
                        )
                        nc.gpsimd.dma_start(
                            outd[:, o0 + olen // 2 : o0 + olen],
                            obuf[:, olen // 2 :],
                        )
                    else:
                        nc.gpsimd.dma_start(outd[:, o0 : o0 + olen], obuf[:])

            pending = None  # (pg, ts) awaiting stage-2
            for pg in range(b_local // PAIR):
                c0 = pg * PAIR
                k = next(i for i in range(len(CHUNKS))
                         if chunk_base[i] <= c0 < chunk_base[i + 1])
                xl, xr = xbufs[k % NXBUF]
                if c0 == chunk_base[k] and k + NXBUF - 1 < len(CHUNKS):
                    issue_chunk(k + NXBUF - 1)
                if c0 % OCH == 0:
                    obufs[pg // (OCH // PAIR)] = opool.tile(
                        [D_IN, (OCH // PAIR) * gfree], f16, tag="obuf",
                        name="obuf",
                    )
                tp = pt.tile([D_IN, pfree], f32)
                for jj in range(PAIR):
                    b = c0 - chunk_base[k] + jj
                    dst = tp[:, jj * D_OUT : (jj + 1) * D_OUT]
                    nc.tensor.matmul(
                        dst[0:H, :],
                        xl[:, b, :],
                        wh3[:, 0:D_OUT],
                        start=True,
                        stop=True,
                        tile_position=(0, 0),
                    )
                    s = b % 2
                    rhs = (wh3[0:H, 2 * D_OUT : 3 * D_OUT] if s == 0
                           else wh3[H:D_IN, 0:D_OUT])
                    nc.tensor.matmul(
                        dst[H:D_IN, :],
                        xr[s * H : (s + 1) * H, b // 2, :],
                        rhs,
                        start=True,
                        stop=True,
                        tile_position=(s * H, H),
                    )
                ts = tpool.tile([D_IN, pfree], f16, tag="ts")
                # split the copy across both engines to halve its latency
                ceng = (nc.vector.tensor_copy, nc.scalar.copy)
                e0, e1 = ceng if pg % 2 == 0 else ceng[::-1]
                e0(ts[:, 0 : pfree // 2], tp[:, 0 : pfree // 2])
                e1(ts[:, pfree // 2 :], tp[:, pfree // 2 :])
                if pending is not None:
                    emit_stage2(*pending)
                pending = (pg, ts)
            emit_stage2(*pending)

    nc.compile()
    return nc


def _get_nc(b_local):
    if b_local not in _CACHE:
        _CACHE[b_local] = _build_nc(b_local)
    return _CACHE[b_local]


def _host_prep(W):
    """Derive the sign diagonal of the reference's QR and the masked W.

    Returns (wm, d) or (None, None) when W doesn't have orthonormal rows
    (then the closed form doesn't apply and the caller falls back)."""
    W = np.ascontiguousarray(W, dtype=np.float32)
    q, _ = np.linalg.qr(W.T)
    d = np.sign((q.T * W).sum(axis=1)).astype(np.float32)
    d[d == 0] = 1.0
    if np.abs(q.T - d[:, None] * W).max() >= 1e-4:
        return None, None
    wm = W * (d > 0).astype(np.float32)[:, None]
    return wm, d


def _reference_fallback(X, W):
    """Faithful numpy port of the reference (QR + eigh) — only used if the
    input W unexpectedly doesn't have orthonormal rows."""
    q, _ = np.linalg.qr(W.T.astype(np.float32))
    w_st = q.T
    y = np.einsum("mi,bij->bmj", w_st, X, optimize=True) @ W.T
    m = 0.5 * (y + y.transpose(0, 2, 1))
    lam, u = np.linalg.eigh(m)
    lam = np.maximum(lam, EPS)
    return np.einsum("bik,bk,bjk->bij", u, lam, u, optimize=True).astype(np.float32)


def run(X, W, trace=False, **trace_kwargs):
    X = np.ascontiguousarray(X, dtype=np.float32)
    wm, d = _host_prep(W)
    if wm is None:
        return _reference_fallback(X, W), None

    wh = wm.T.astype(np.float16)  # [128, 64] = W^T
    whx = np.concatenate([wh[H:D_IN], wh[H:D_IN]], axis=0)  # [128, 64]
    wh3 = np.concatenate([wh, wh, whx], axis=1)  # [128, 192]

    # [B, i, b, j] i-major fp16
    xh = X.astype(np.float16)
    xh = xh.reshape(N_CORES, B_LOCAL, D_IN, D_IN).transpose(0, 2, 1, 3)
    # left block-column of L' (top 64x64 block halved; exact in fp16)
    xl = np.ascontiguousarray(xh[:, :, :, 0:H])
    xl[:, 0:H, :, :] *= np.float16(0.5)
    xl = xl.reshape(N_CORES, D_IN, B_LOCAL * H)
    # right-bottom block X22/2, item-pair packed onto 128 partitions
    xr = xh[:, H:D_IN, :, H:D_IN] * np.float16(0.5)  # [core, 64, b, 64]
    xr = xr.reshape(N_CORES, H, B_LOCAL // 2, 2, H).transpose(0, 3, 1, 2, 4)
    xr = np.ascontiguousarray(xr).reshape(N_CORES, D_IN, (B_LOCAL // 2) * H)

    from concourse.bass_utils import run_bass_kernel_spmd

    nc = _get_nc(B_LOCAL)
    in_maps = [
        {"XL": xl[c], "XR": xr[c], "WH3": wh3} for c in range(N_CORES)
    ]
    last_err = None
    for _attempt in range(3):
        try:
            res = run_bass_kernel_spmd(
                nc, in_maps, list(range(N_CORES)), trace=trace, **trace_kwargs
            )
            break
        except Exception as e:  # noqa: BLE001 - transient NRT device errors
            last_err = e
            import time

            time.sleep(2.0)
    else:
        raise last_err

    n_pair = B_LOCAL // PAIR
    z = np.empty((B_TOTAL, D_OUT, D_OUT), dtype=np.float32)
    for c in range(N_CORES):
        o = res.results[c]["OUT"].reshape(2, D_OUT, n_pair, GROUP, D_OUT)
        # o[h, m, pg, j, n] = Z[pg*16 + h*8 + j][m, n]
        o = o.transpose(2, 0, 3, 1, 4).reshape(B_LOCAL, D_OUT, D_OUT)
        z[c * B_LOCAL : (c + 1) * B_LOCAL] = o
    out = z + z.transpose(0, 2, 1)  # S = Z + Z^T (L' + L'^T = X)
    neg = d < 0
    if neg.any():
        idx = np.where(neg)[0]
        out[:, idx, idx] += EPS
    return out, res


def kernel(X, W):
    return run(X, W)[0]


# revision 15
# speedup vs baseline: 3.2814x; 3.2814x over previous
"""
nn_BiReBlock kernel for 8x Trainium2 NeuronCores.

Mathematical reduction (same as the verified baseline)
------------------------------------------------------
reference(X, W) with W having orthonormal rows reduces to
    out = Wm @ X @ Wm^T + eps * diag(1_N)
where Wm = W with QR-sign-negative rows zeroed (for the actual seed-0 W,
QR reproduces W exactly so Wm = W, N = {}).

Device computation (v6, "packed half-staircase")
------------------------------------------------
The kernel is HBM-DMA-bound, so we ship as few bytes as possible:

* fp16 is plenty (2e-2 budget vs ~4e-4 measured end-to-end), no residual.
* X is symmetric, so only the lower 2x2 block staircase of it is shipped:
  L' = [[X11/2, 0], [X21, X22/2]], which satisfies L' + L'^T = X.
  Since S = Wm X Wm^T is symmetric the device computes
      Z_b = Wm L'_b^T Wm^T
  and the host reconstructs S = Z + Z^T for free.  75% of X shipped.
* The left block-column [128, 64] is one dense SBUF tile per item.
  The right-bottom block [64, 64] of consecutive item pairs is packed
  onto all 128 SBUF partitions (even item on 0:64, odd on 64:128), so
  every DMA runs with full 128-partition parallelism and no SBUF zeros
  are ever needed.  Stage 1 = 2 column-tiled matmuls per item:
    V[0:64]  = (left col)^T W^T        K=128, tile (0,0)
    V[64:128]= (X22/2)^T  W^T[64:128]  K=64,  tile (0|64, 64)
  where the odd-item MM contracts on partitions 64:128 against the
  resident W^T rows, and the even-item MM contracts on partitions 0:64
  against a shifted copy of W^T rows 64:128 (third section of WH3).
* stage 2 packs two 8-item groups into PSUM partitions 0:64 / 64:128
  via column tiling; it is emitted one pair late (software pipelining)
  so its wait for the PSUM->SBUF copy never stalls the in-order PE.
* PSUM->SBUF copies convert to fp16 and are split/alternated across
  the Vector and Scalar engines; output is fp16.

HBM traffic/core: 12 MB X + 4 MB out (vs 32 MB baseline).
"""

import numpy as np

B_TOTAL = 4096
N_CORES = 8
B_LOCAL = B_TOTAL // N_CORES
D_IN = 128
D_OUT = 64
EPS = 1e-4

_CACHE = {}

CHUNKS = [32, 32] + [64] * 7
assert sum(CHUNKS) == B_LOCAL
XCH_MAX = max(CHUNKS)
NXBUF = 6
GROUP = 8
PAIR = 2 * GROUP          # items per packed stage-2 PSUM bank
OCH = 64                  # items per output flush
H = D_IN // 2             # 64


def _build_nc(b_local):
    import concourse.tile as tile
    from concourse import bacc, mybir

    f32 = mybir.dt.float32
    f16 = mybir.dt.float16
    nc = bacc.Bacc(None, target_bir_lowering=False)

    # left block-column of L', i-major: XL[i, b*64 + j] = L'_b[i, j]
    xld = nc.dram_tensor("XL", [D_IN, b_local * H], f16, kind="ExternalInput")
    # right-bottom blocks, item-pair packed:
    # XR[s*64 + p, q*64 + j] = (X22/2)_{2q+s}[p, j]
    xrd = nc.dram_tensor("XR", [D_IN, (b_local // 2) * H], f16,
                         kind="ExternalInput")
    # [W^T | W^T | [W^T[64:]; 0] | [0; W^T[64:]]] — the zero halves of the
    # last two sections mask out the other item of each packed XR pair
    wd = nc.dram_tensor("WH3", [D_IN, 4 * D_OUT], f16, kind="ExternalInput")
    n_pair = b_local // PAIR
    outd = nc.dram_tensor("OUT", [D_IN, n_pair * GROUP * D_OUT], f16,
                          kind="ExternalOutput")

    gfree = GROUP * D_OUT   # 512
    pfree = PAIR * D_OUT    # 1024

    with tile.TileContext(nc) as tc:
        with (
            tc.tile_pool(name="const", bufs=1) as cpool,
            tc.tile_pool(name="tsb", bufs=3) as tpool,
            tc.tile_pool(name="obuf", bufs=2) as opool,
            tc.tile_pool(name="psum_t", bufs=3, space="PSUM") as pt,
            tc.tile_pool(name="psum_s", bufs=2, space="PSUM") as ps,
        ):
            wh3 = cpool.tile([D_IN, 4 * D_OUT], f16)
            nc.sync.dma_start(wh3[:], wd[:])

            # fixed X tiles (NXBUF buffers x {left, right}), rotated manually
            xbufs = [
                (cpool.tile([D_IN, XCH_MAX, H], f16, name=f"xl{i}"),
                 cpool.tile([D_IN, XCH_MAX // 2, H], f16, name=f"xr{i}"))
                for i in range(NXBUF)
            ]

            chunk_base = np.cumsum([0] + CHUNKS).tolist()

            def issue_chunk(k):
                b0, n = chunk_base[k], CHUNKS[k]
                xl, xr = xbufs[k % NXBUF]
                nc.sync.dma_start(
                    xl[:, 0:n, :], xld[:, b0 * H : (b0 + n) * H]
                )
                nc.sync.dma_start(
                    xr[:, 0 : n // 2, :],
                    xrd[:, (b0 // 2) * H : ((b0 + n) // 2) * H],
                )

            # chunk k+NXBUF-1 is issued only once chunk k's compute emission
            # begins, so WAR edges against the buffer's previous user are
            # ordered correctly.
            for k in range(min(NXBUF - 1, len(CHUNKS))):
                issue_chunk(k)

            obufs = {}

            def emit_stage2(pg, ts):
                sp = ps.tile([D_IN, gfree], f32, tag="sp", name="sp")
                for h in range(2):
                    nc.tensor.matmul(
                        sp[h * D_OUT : (h + 1) * D_OUT, :],
                        wh3[:, h * D_OUT : (h + 1) * D_OUT],
                        ts[:, h * gfree : (h + 1) * gfree],
                        start=True,
                        stop=True,
                        tile_position=(0, h * D_OUT),
                    )
                obuf = obufs[pg // (OCH // PAIR)]
                off = (pg % (OCH // PAIR)) * gfree
                scpy = nc.scalar.copy if pg % 2 == 0 else nc.vector.tensor_copy
                scpy(obuf[:, off : off + gfree], sp[:])
                c0 = pg * PAIR
                if (c0 + PAIR) % OCH == 0:
                    o0 = (pg // (OCH // PAIR)) * (OCH // PAIR) * gfree
                    olen = (OCH // PAIR) * gfree
                    if c0 + PAIR == b_local:
                        nc.gpsimd.dma_start(
                            outd[:, o0 : o0 + olen // 2], obuf[:, : olen // 2]
                        )
                        nc.gpsimd.dma_start(
                            outd[:, o0 + olen // 2 : o0 + olen],
                            obuf[:, olen // 2 :],
                        )
                    else:
                        nc.gpsimd.dma_start(outd[:, o0 : o0 + olen], obuf[:])

            pending = None  # (pg, ts) awaiting stage-2
            for pg in range(b_local // PAIR):
                c0 = pg * PAIR
                k = next(i for i in range(len(CHUNKS))
                         if chunk_base[i] <= c0 < chunk_base[i + 1])
                xl, xr = xbufs[k % NXBUF]
                if c0 == chunk_base[k] and k + NXBUF - 1 < len(CHUNKS):
                    issue_chunk(k + NXBUF - 1)
                if c0 % OCH == 0:
                    obufs[pg // (OCH // PAIR)] = opool.tile(
                        [D_IN, (OCH // PAIR) * gfree], f16, tag="obuf",
                        name="obuf",
                    )
                tp = pt.tile([D_IN, pfree], f32)
                for jj in range(PAIR):
                    b = c0 - chunk_base[k] + jj
                    dst = tp[:, jj * D_OUT : (jj + 1) * D_OUT]
                    nc.tensor.matmul(
                        dst[0:H, :],
                        xl[:, b, :],
                        wh3[:, 0:D_OUT],
                        start=True,
                        stop=True,
                        tile_position=(0, 0),
                    )
                    s = b % 2
                    nc.tensor.matmul(
                        dst[H:D_IN, :],
                        xr[:, b // 2, :],
                        wh3[:, (2 + s) * D_OUT : (3 + s) * D_OUT],
                        start=True,
                        stop=True,
                        tile_position=(0, H),
                    )
                ts = tpool.tile([D_IN, pfree], f16, tag="ts")
                # split the copy across both engines to halve its latency
                ceng = (nc.vector.tensor_copy, nc.scalar.copy)
                e0, e1 = ceng if pg % 2 == 0 else ceng[::-1]
                e0(ts[:, 0 : pfree // 2], tp[:, 0 : pfree // 2])
                e1(ts[:, pfree // 2 :], tp[:, pfree // 2 :])
                if pending is not None:
                    emit_stage2(*pending)
                pending = (pg, ts)
            emit_stage2(*pending)

    nc.compile()
    return nc


def _get_nc(b_local):
    if b_local not in _CACHE:
        _CACHE[b_local] = _build_nc(b_local)
    return _CACHE[b_local]


def _host_prep(W):
    """Derive the sign diagonal of the reference's QR and the masked W.

    Returns (wm, d) or (None, None) when W doesn't have orthonormal rows
    (then the closed form doesn't apply and the caller falls back)."""
    W = np.ascontiguousarray(W, dtype=np.float32)
    q, _ = np.linalg.qr(W.T)
    d = np.sign((q.T * W).sum(axis=1)).astype(np.float32)
    d[d == 0] = 1.0
    if np.abs(q.T - d[:, None] * W).max() >= 1e-4:
        return None, None
    wm = W * (d > 0).astype(np.float32)[:, None]
    return wm, d


def _reference_fallback(X, W):
    """Faithful numpy port of the reference (QR + eigh) — only used if the
    input W unexpectedly doesn't have orthonormal rows."""
    q, _ = np.linalg.qr(W.T.astype(np.float32))
    w_st = q.T
    y = np.einsum("mi,bij->bmj", w_st, X, optimize=True) @ W.T
    m = 0.5 * (y + y.transpose(0, 2, 1))
    lam, u = np.linalg.eigh(m)
    lam = np.maximum(lam, EPS)
    return np.einsum("bik,bk,bjk->bij", u, lam, u, optimize=True).astype(np.float32)


def run(X, W, trace=False, **trace_kwargs):
    X = np.ascontiguousarray(X, dtype=np.float32)
    wm, d = _host_prep(W)
    if wm is None:
        return _reference_fallback(X, W), None

    wh = wm.T.astype(np.float16)  # [128, 64] = W^T
    zz = np.zeros((H, D_OUT), dtype=np.float16)
    whe = np.concatenate([wh[H:D_IN], zz], axis=0)  # masks odd item
    who = np.concatenate([zz, wh[H:D_IN]], axis=0)  # masks even item
    wh3 = np.concatenate([wh, wh, whe, who], axis=1)  # [128, 256]

    # [B, i, b, j] i-major fp16
    xh = X.astype(np.float16)
    xh = xh.reshape(N_CORES, B_LOCAL, D_IN, D_IN).transpose(0, 2, 1, 3)
    # left block-column of L' (top 64x64 block halved; exact in fp16)
    xl = np.ascontiguousarray(xh[:, :, :, 0:H])
    xl[:, 0:H, :, :] *= np.float16(0.5)
    xl = xl.reshape(N_CORES, D_IN, B_LOCAL * H)
    # right-bottom block X22/2, item-pair packed onto 128 partitions
    xr = xh[:, H:D_IN, :, H:D_IN] * np.float16(0.5)  # [core, 64, b, 64]
    xr = xr.reshape(N_CORES, H, B_LOCAL // 2, 2, H).transpose(0, 3, 1, 2, 4)
    xr = np.ascontiguousarray(xr).reshape(N_CORES, D_IN, (B_LOCAL // 2) * H)

    from concourse.bass_utils import run_bass_kernel_spmd

    nc = _get_nc(B_LOCAL)
    in_maps = [
        {"XL": xl[c], "XR": xr[c], "WH3": wh3} for c in range(N_CORES)
    ]
    last_err = None
    for _attempt in range(3):
        try:
            res = run_bass_kernel_spmd(
                nc, in_maps, list(range(N_CORES)), trace=trace, **trace_kwargs
            )
            break
        except Exception as e:  # noqa: BLE001 - transient NRT device errors
            last_err = e
            import time

            time.sleep(2.0)
    else:
        raise last_err

    n_pair = B_LOCAL // PAIR
    z = np.empty((B_TOTAL, D_OUT, D_OUT), dtype=np.float32)
    for c in range(N_CORES):
        o = res.results[c]["OUT"].reshape(2, D_OUT, n_pair, GROUP, D_OUT)
        # o[h, m, pg, j, n] = Z[pg*16 + h*8 + j][m, n]
        o = o.transpose(2, 0, 3, 1, 4).reshape(B_LOCAL, D_OUT, D_OUT)
        z[c * B_LOCAL : (c + 1) * B_LOCAL] = o
    out = z + z.transpose(0, 2, 1)  # S = Z + Z^T (L' + L'^T = X)
    neg = d < 0
    if neg.any():
        idx = np.where(neg)[0]
        out[:, idx, idx] += EPS
    return out, res


def kernel(X, W):
    return run(X, W)[0]
